# revision 1
# baseline (speedup 1.0000x reference)
"""Trainium2 Bass kernel for CrossAttention (B=2, N=2048, C=1024, H=16, D=64).

Sharding: 8 cores = 2 (batch) x 4 (head groups of 4 heads).
Each core computes q/k/v projections for its 4 heads, full N x N attention
(head-local), and a partial output projection over its 256 contraction dims.
Host sums the 4 partials per batch element and adds the bias.

Layout strategy (all on-chip, no transposes needed on device):
  - host feeds x^T, y^T  [C, N] so the C contraction sits on partitions
  - qT/kT computed in [head-pair dims(128), N] layout (S^T matmul operands)
  - v computed in natural [N, 4*65] layout with a ones column per head
    (augmented-V: AV matmul emits o^T rows 0..63 and the softmax rowsum
    in row 64 of the same PSUM tile, for free)
  - softmax has no max-subtraction: scores are ~N(0,1), exp is safe in fp32
  - 1/rowsum broadcast across 64 partitions via a k=64 block-diag ones matmul
  - matmuls run in float32r (4x faster than fp32; ~1.5e-4 scale-rel error)
"""

import sys
import numpy as np

for _p in ("/opt/trn_rl_repo",):
    if _p not in sys.path:
        sys.path.insert(0, _p)

B, N, C, H = 2, 2048, 1024, 16
D = C // H          # 64
HPC = 4             # heads per core
G = H // HPC        # 4 head groups
NCORES = 8
KC = C // 128       # 8 contraction chunks
TC = 4              # token chunks of 512
QC = 4              # query chunks of 512
KB = N // 128       # 16 key blocks
QB = N // 128       # 16 query blocks (for proj)

_CACHE = {}


def _build():
    import concourse.bacc as bacc
    import concourse.mybir as mybir
    import concourse.tile as tile

    fr = mybir.dt.float32r
    f32 = mybir.dt.float32
    Exp = mybir.ActivationFunctionType.Exp

    nc = bacc.Bacc("TRN2", target_bir_lowering=False, debug=False,
                   num_devices=NCORES)

    xT = nc.dram_tensor("xT", [C, N], fr, kind="ExternalInput")
    yT = nc.dram_tensor("yT", [C, N], fr, kind="ExternalInput")
    wq = nc.dram_tensor("wq", [C, HPC * D], fr, kind="ExternalInput")
    wk = nc.dram_tensor("wk", [C, HPC * D], fr, kind="ExternalInput")
    wv = nc.dram_tensor("wv", [C, HPC * D], fr, kind="ExternalInput")
    wp = nc.dram_tensor("wp", [HPC * D, C], fr, kind="ExternalInput")
    bd = nc.dram_tensor("bd", [64, 128], fr, kind="ExternalInput")
    out = nc.dram_tensor("out", [N, C], f32, kind="ExternalOutput")

    xT3 = xT.rearrange("(ko ki) t -> ki ko t", ki=128)
    yT3 = yT.rearrange("(ko ki) t -> ki ko t", ki=128)
    wq3 = wq.rearrange("(ko ki) m -> ki ko m", ki=128)
    wk3 = wk.rearrange("(ko ki) m -> ki ko m", ki=128)
    wv3 = wv.rearrange("(ko ki) m -> ki ko m", ki=128)
    wp3 = wp.rearrange("(po pi) f -> pi po f", pi=128)

    with tile.TileContext(nc) as tc:
        import contextlib
        with contextlib.ExitStack() as ctx:
            sb_w = ctx.enter_context(tc.tile_pool(name="sb_w", bufs=1))
            sb_x = ctx.enter_context(tc.tile_pool(name="sb_x", bufs=1))
            sb_qk = ctx.enter_context(tc.tile_pool(name="sb_qk", bufs=1))
            sb_e = ctx.enter_context(tc.tile_pool(name="sb_e", bufs=3))
            sb_on = ctx.enter_context(tc.tile_pool(name="sb_on", bufs=1))
            sb_r = ctx.enter_context(tc.tile_pool(name="sb_r", bufs=2))
            sb_out = ctx.enter_context(tc.tile_pool(name="sb_out", bufs=2))
            ps_a = ctx.enter_context(tc.tile_pool(name="ps_a", bufs=2, space="PSUM"))

            # ---- constants ----
            ones_f = sb_w.tile([128, 4], f32, tag="ones_f")
            nc.vector.memset(ones_f[:], 1.0)
            onesbd = sb_w.tile([64, 128], fr, tag="onesbd")
            nc.sync.dma_start(out=onesbd[:], in_=bd[:])
            rr_f = sb_w.tile([64, 512], f32, tag="rr_f")
            nc.vector.memset(rr_f[:], 0.0)
            rinv_ab = [sb_w.tile([64, 512], fr, tag=f"rinv{i}",
                                 name=f"rinv{i}") for i in range(2)]
            for i in range(2):
                nc.vector.tensor_copy(rinv_ab[i][:], rr_f[:])

            # ---- load weights + inputs ----
            twq = sb_w.tile([128, KC, HPC * D], fr, tag="twq")
            nc.sync.dma_start(out=twq[:], in_=wq3[:])
            twk = sb_w.tile([128, KC, HPC * D], fr, tag="twk")
            nc.sync.dma_start(out=twk[:], in_=wk3[:])
            twv = sb_w.tile([128, KC, HPC * D], fr, tag="twv")
            nc.sync.dma_start(out=twv[:], in_=wv3[:])
            twp = sb_w.tile([128, 2, C], fr, tag="twp")
            nc.sync.dma_start(out=twp[:], in_=wp3[:])

            # persistent activations
            qT = [sb_qk.tile([128, N], fr, tag=f"qT{p}", name=f"qT{p}") for p in range(2)]
            kT = [sb_qk.tile([128, N], fr, tag=f"kT{p}", name=f"kT{p}") for p in range(2)]
            tv = sb_qk.tile([128, KB, HPC * 65], fr, tag="tv")
            onorm = [sb_on.tile([128, N], fr, tag=f"onorm{p}", name=f"onorm{p}") for p in range(2)]

            # ---- Phase A: y chunks -> k (both pairs) + v ----
            for t in range(TC):
                tsl = slice(t * 512, (t + 1) * 512)
                yc = sb_x.tile([128, KC, 512], fr, tag="yc")
                nc.sync.dma_start(out=yc[:], in_=yT3[:, :, tsl])
                for p in range(2):
                    psl = slice(p * 128, (p + 1) * 128)
                    pk = ps_a.tile([128, 512], f32, tag="qkv")
                    for kc in range(KC):
                        nc.tensor.matmul(
                            pk[:], twk[:, kc, psl], yc[:, kc, :],
                            start=(kc == 0), stop=(kc == KC - 1))
                    nc.vector.tensor_copy(kT[p][:, tsl], pk[:])
                for j in range(4):
                    kb = 4 * t + j
                    pv = ps_a.tile([128, 512], f32, tag="qkv")
                    for kc in range(KC):
                        nc.tensor.matmul(
                            pv[:, 0:HPC * D],
                            yc[:, kc, j * 128:(j + 1) * 128], twv[:, kc, :],
                            start=(kc == 0), stop=(kc == KC - 1))
                    dst = bass_ap_heads(tv, kb)
                    nc.vector.tensor_copy(dst, pv[:, 0:HPC * D].rearrange(
                        "p (h d) -> p h d", h=HPC))
                    nc.vector.tensor_copy(tv[:, kb, 64::65], ones_f[:])

            def q_proj(t):
                tsl = slice(t * 512, (t + 1) * 512)
                xc = sb_x.tile([128, KC, 512], fr, tag="xc", name=f"xc{t}")
                nc.sync.dma_start(out=xc[:], in_=xT3[:, :, tsl])
                for p in range(2):
                    psl = slice(p * 128, (p + 1) * 128)
                    pq = ps_a.tile([128, 512], f32, tag="qkv",
                                   name=f"pq{p}_{t}")
                    for kc in range(KC):
                        nc.tensor.matmul(
                            pq[:], twq[:, kc, psl], xc[:, kc, :],
                            start=(kc == 0), stop=(kc == KC - 1))
                    nc.vector.tensor_copy(qT[p][:, tsl], pq[:])

            def attn_chunk(p, qc, ps_s, ps_av, rinv):
                qsl = slice(qc * 512, (qc + 1) * 512)
                av = [ps_av.tile([65, 512], f32, tag=f"av{h}",
                                 name=f"av{h}_{p}_{qc}") for h in range(2)]
                for kb in range(KB):
                    s_ps = ps_s.tile([128, 1024], f32, tag="s",
                                     name=f"s_{p}_{qc}_{kb}")
                    nc.tensor.matmul(
                        s_ps[:, 0:512],
                        kT[p][0:64, kb * 128:(kb + 1) * 128],
                        qT[p][0:64, qsl], start=True, stop=True)
                    nc.tensor.matmul(
                        s_ps[:, 512:1024],
                        kT[p][64:128, kb * 128:(kb + 1) * 128],
                        qT[p][64:128, qsl], start=True, stop=True)
                    e = sb_e.tile([128, 1024], fr, tag="e",
                                  name=f"e_{p}_{qc}_{kb}")
                    nc.scalar.activation(e[:], s_ps[:], Exp, scale=0.125)
                    for h in range(2):
                        hh = (2 * p + h) * 65
                        nc.tensor.matmul(
                            av[h][:], tv[:, kb, hh:hh + 65],
                            e[:, h * 512:(h + 1) * 512],
                            start=(kb == 0), stop=(kb == KB - 1))
                # normalize: rinv rows -> k=64 bcast matmul -> TT-mul
                with nc.allow_low_precision(reason="softmax denom recip"):
                    for h in range(2):
                        nc.vector.reciprocal(rinv[32 * h:32 * h + 1, :],
                                             av[h][64:65, :])
                r_ps = ps_a.tile([128, 512], f32, tag="qkv",
                                 name=f"rps_{p}_{qc}")
                nc.tensor.matmul(r_ps[:], onesbd[:], rinv[:],
                                 start=True, stop=True)
                r_sb = sb_r.tile([128, 512], f32, tag="r_sb",
                                 name=f"rsb_{p}_{qc}")
                nc.vector.tensor_copy(r_sb[:], r_ps[:])
                for h in range(2):
                    nc.vector.tensor_mul(
                        onorm[p][h * 64:(h + 1) * 64, qsl],
                        av[h][0:64, :], r_sb[h * 64:(h + 1) * 64, :])

            def proj_block(qb):
                bsl = slice(qb * 128, (qb + 1) * 128)
                so = sb_out.tile([128, 1024], f32, tag="so", name=f"so{qb}")
                for cc in range(2):
                    csl = slice(cc * 512, (cc + 1) * 512)
                    pp = ps_a.tile([128, 512], f32, tag="qkv",
                                   name=f"pp{qb}_{cc}")
                    nc.tensor.matmul(pp[:], onorm[0][:, bsl], twp[:, 0, csl],
                                     start=True, stop=False)
                    nc.tensor.matmul(pp[:], onorm[1][:, bsl], twp[:, 1, csl],
                                     start=False, stop=True)
                    nc.vector.tensor_copy(so[:, csl], pp[:])
                nc.sync.dma_start(out=out[bsl, :], in_=so[:])

            # ---- Phase B/C: q-proj chunks interleaved with attention ----
            with tc.tile_pool(name="ps_s", bufs=2, space="PSUM") as ps_s, \
                 tc.tile_pool(name="ps_av", bufs=1, space="PSUM") as ps_av:
                for qc in range(QC):
                    q_proj(qc)
                    attn_chunk(0, qc, ps_s, ps_av, rinv_ab[qc % 2])
                for qc in range(QC):
                    attn_chunk(1, qc, ps_s, ps_av, rinv_ab[qc % 2])
                    for qb in range(4 * qc, 4 * qc + 4):
                        proj_block(qb)

    nc.finalize()
    return nc


def bass_ap_heads(tv, kb):
    # view of tv[:, kb, :] as [128, HPC, 64] hitting cols h*65..h*65+63
    return tv[:, kb, :].rearrange("p (h s) -> p h s", h=HPC)[:, :, 0:64]


def _shard_inputs(x, y, Wq, Wkv, Wp):
    x = np.asarray(x, dtype=np.float32)
    y = np.asarray(y, dtype=np.float32)
    Wq = np.asarray(Wq, dtype=np.float32)
    Wkv = np.asarray(Wkv, dtype=np.float32)
    Wp = np.asarray(Wp, dtype=np.float32)
    bd = np.zeros((64, 128), dtype=np.float32)
    bd[0, 0:64] = 1.0
    bd[32, 64:128] = 1.0
    in_maps = []
    for core in range(NCORES):
        b, g = divmod(core, G)
        sl = slice(g * HPC * D, (g + 1) * HPC * D)
        in_maps.append({
            "xT": np.ascontiguousarray(x[b].T),
            "yT": np.ascontiguousarray(y[b].T),
            "wq": np.ascontiguousarray(Wq[sl, :].T),
            "wk": np.ascontiguousarray(Wkv[sl, :].T),
            "wv": np.ascontiguousarray(Wkv[C:][sl, :].T),
            "wp": np.ascontiguousarray(Wp[:, sl].T),
            "bd": bd,
        })
    return in_maps


def kernel(x, y, Wq, Wkv, Wp, bp):
    from concourse.bass_utils import run_bass_kernel_spmd

    if "nc" not in _CACHE:
        _CACHE["nc"] = _build()
    nc = _CACHE["nc"]

    in_maps = _shard_inputs(x, y, Wq, Wkv, Wp)
    res = run_bass_kernel_spmd(nc, in_maps, core_ids=list(range(NCORES)))

    bp = np.asarray(bp, dtype=np.float32)
    full = np.zeros((B, N, C), dtype=np.float32)
    for core in range(NCORES):
        b = core // G
        full[b] += res.results[core]["out"]
    full += bp[None, None, :]
    return full



# revision 2
# speedup vs baseline: 12.2167x; 12.2167x over previous
"""Trainium2 Bass kernel for CrossAttention (B=2, N=2048, C=1024, H=16, D=64).

Sharding: 8 cores = 2 (batch) x 4 (head groups of 4 heads).
Each core computes q/k/v projections for its 4 heads, full N x N attention
(head-local), and a partial output projection over its 256 contraction dims.
Host sums the 4 partials per batch element and adds the bias.

Layout strategy (all on-chip, no transposes needed on device):
  - host feeds x^T, y^T  [C, N] so the C contraction sits on partitions
  - qT/kT computed in [head-pair dims(128), N] layout (S^T matmul operands)
  - v computed in natural [N, 4*65] layout with a ones column per head
    (augmented-V: AV matmul emits o^T rows 0..63 and the softmax rowsum
    in row 64 of the same PSUM tile, for free)
  - softmax has no max-subtraction: scores are ~N(0,1), exp is safe in fp32
  - 1/rowsum broadcast across 64 partitions via a k=64 block-diag ones matmul
  - matmuls run in float32r (4x faster than fp32; ~1.5e-4 scale-rel error)

v2 perf structure (vs the v1 baseline that measured 372.7us on HW):
  - input chunks double/triple-buffered (sb_in bufs=3) so chunk DMA overlaps
    projection matmuls instead of stalling the PE ~10us per chunk
  - AV PSUM is evacuated to SBUF immediately (o rows + rowsum row), freeing
    the accumulator banks ~2us after the last AV matmul; softmax
    normalization (recip + broadcast + mul) runs off the critical path
  - the softmax denominator uses one batched reciprocal_approx_fast on
    [33,512] (rows 0/32 live, filler rows preset to 1.0) instead of two
    serial single-partition exact reciprocals (6.7us -> 0.7us)
  - q_proj(0) is issued before the kv phase so the attention/exp pipeline
    (ScalarE is the steady-state bottleneck) ramps ~30us earlier
"""

import sys
import numpy as np

for _p in ("/opt/trn_rl_repo",):
    if _p not in sys.path:
        sys.path.insert(0, _p)

B, N, C, H = 2, 2048, 1024, 16
D = C // H          # 64
HPC = 4             # heads per core
G = H // HPC        # 4 head groups
NCORES = 8
KC = C // 128       # 8 contraction chunks
TC = 4              # token chunks of 512
QC = 4              # query chunks of 512
KB = N // 128       # 16 key blocks
QB = N // 128       # 16 query blocks (for proj)

_CACHE = {}


def _build():
    import concourse.bacc as bacc
    import concourse.mybir as mybir
    import concourse.tile as tile

    fr = mybir.dt.float32r
    f32 = mybir.dt.float32
    Exp = mybir.ActivationFunctionType.Exp

    nc = bacc.Bacc("TRN2", target_bir_lowering=False, debug=False,
                   num_devices=NCORES)

    xT = nc.dram_tensor("xT", [C, N], fr, kind="ExternalInput")
    yT = nc.dram_tensor("yT", [C, N], fr, kind="ExternalInput")
    wq = nc.dram_tensor("wq", [C, HPC * D], fr, kind="ExternalInput")
    wk = nc.dram_tensor("wk", [C, HPC * D], fr, kind="ExternalInput")
    wv = nc.dram_tensor("wv", [C, HPC * D], fr, kind="ExternalInput")
    wp = nc.dram_tensor("wp", [HPC * D, C], fr, kind="ExternalInput")
    bd = nc.dram_tensor("bd", [64, 128], f32, kind="ExternalInput")
    out = nc.dram_tensor("out", [N, C], f32, kind="ExternalOutput")

    xT3 = xT.rearrange("(ko ki) t -> ki ko t", ki=128)
    yT3 = yT.rearrange("(ko ki) t -> ki ko t", ki=128)
    wq3 = wq.rearrange("(ko ki) m -> ki ko m", ki=128)
    wk3 = wk.rearrange("(ko ki) m -> ki ko m", ki=128)
    wv3 = wv.rearrange("(ko ki) m -> ki ko m", ki=128)
    wp3 = wp.rearrange("(po pi) f -> pi po f", pi=128)

    with tile.TileContext(nc) as tc:
        import contextlib
        with contextlib.ExitStack() as ctx:
            sb_w = ctx.enter_context(tc.tile_pool(name="sb_w", bufs=1))
            sb_in = ctx.enter_context(tc.tile_pool(name="sb_in", bufs=3))
            sb_qk = ctx.enter_context(tc.tile_pool(name="sb_qk", bufs=1))
            sb_e = ctx.enter_context(tc.tile_pool(name="sb_e", bufs=3))
            sb_on = ctx.enter_context(tc.tile_pool(name="sb_on", bufs=1))
            sb_r = ctx.enter_context(tc.tile_pool(name="sb_r", bufs=2))
            sb_rc = ctx.enter_context(tc.tile_pool(name="sb_rc", bufs=1))
            sb_out = ctx.enter_context(tc.tile_pool(name="sb_out", bufs=2))
            ps_s = ctx.enter_context(tc.tile_pool(name="ps_s", bufs=2, space="PSUM"))
            ps_av = ctx.enter_context(tc.tile_pool(name="ps_av", bufs=1, space="PSUM"))
            ps_a = ctx.enter_context(tc.tile_pool(name="ps_a", bufs=2, space="PSUM"))

            # ---- constants ----
            ones_f = sb_w.tile([128, 4], f32, tag="ones_f")
            nc.vector.memset(ones_f[:], 1.0)
            onesbd = sb_w.tile([64, 128], f32, tag="onesbd")
            nc.sync.dma_start(out=onesbd[:], in_=bd[:])
            # reciprocal staging: rows 0/32 carry the two heads' rowsums,
            # all other rows preset to 1.0 (recip(1)=1, and the broadcast
            # matmul multiplies them by bd=0 anyway)
            rcin = [sb_rc.tile([64, 512], f32, tag=f"rcin{i}",
                               name=f"rcin{i}") for i in range(2)]
            rcout = [sb_rc.tile([64, 512], f32, tag=f"rcout{i}",
                                name=f"rcout{i}") for i in range(2)]
            for i in range(2):
                nc.vector.memset(rcin[i][:], 1.0)
                nc.vector.memset(rcout[i][:], 1.0)

            # ---- load weights ----
            twq = sb_w.tile([128, KC, HPC * D], fr, tag="twq")
            nc.sync.dma_start(out=twq[:], in_=wq3[:])
            twk = sb_w.tile([128, KC, HPC * D], fr, tag="twk")
            nc.sync.dma_start(out=twk[:], in_=wk3[:])
            twv = sb_w.tile([128, KC, HPC * D], fr, tag="twv")
            nc.sync.dma_start(out=twv[:], in_=wv3[:])
            twp = sb_w.tile([128, 2, C], fr, tag="twp")
            nc.sync.dma_start(out=twp[:], in_=wp3[:])

            # persistent activations
            qT = [sb_qk.tile([128, N], fr, tag=f"qT{p}", name=f"qT{p}") for p in range(2)]
            kT = [sb_qk.tile([128, N], fr, tag=f"kT{p}", name=f"kT{p}") for p in range(2)]
            tv = sb_qk.tile([128, KB, HPC * 65], fr, tag="tv")
            onorm = [sb_on.tile([128, N], fr, tag=f"onorm{p}", name=f"onorm{p}") for p in range(2)]

            def kv_chunk(t):
                tsl = slice(t * 512, (t + 1) * 512)
                yc = sb_in.tile([128, KC, 512], fr, tag="chunk", name=f"yc{t}")
                nc.sync.dma_start(out=yc[:], in_=yT3[:, :, tsl])
                for p in range(2):
                    psl = slice(p * 128, (p + 1) * 128)
                    pk = ps_a.tile([128, 512], f32, tag="qkv", name=f"pk{p}_{t}")
                    for kc in range(KC):
                        nc.tensor.matmul(
                            pk[:], twk[:, kc, psl], yc[:, kc, :],
                            start=(kc == 0), stop=(kc == KC - 1))
                    nc.vector.tensor_copy(kT[p][:, tsl], pk[:])
                for j in range(4):
                    kb = 4 * t + j
                    pv = ps_a.tile([128, 512], f32, tag="qkv", name=f"pv{t}_{j}")
                    for kc in range(KC):
                        nc.tensor.matmul(
                            pv[:, 0:HPC * D],
                            yc[:, kc, j * 128:(j + 1) * 128], twv[:, kc, :],
                            start=(kc == 0), stop=(kc == KC - 1))
                    dst = bass_ap_heads(tv, kb)
                    nc.vector.tensor_copy(dst, pv[:, 0:HPC * D].rearrange(
                        "p (h d) -> p h d", h=HPC))
                    nc.vector.tensor_copy(tv[:, kb, 64::65], ones_f[:])

            def q_proj(t):
                tsl = slice(t * 512, (t + 1) * 512)
                xc = sb_in.tile([128, KC, 512], fr, tag="chunk", name=f"xc{t}")
                nc.sync.dma_start(out=xc[:], in_=xT3[:, :, tsl])
                for p in range(2):
                    psl = slice(p * 128, (p + 1) * 128)
                    pq = ps_a.tile([128, 512], f32, tag="qkv",
                                   name=f"pq{p}_{t}")
                    for kc in range(KC):
                        nc.tensor.matmul(
                            pq[:], twq[:, kc, psl], xc[:, kc, :],
                            start=(kc == 0), stop=(kc == KC - 1))
                    nc.vector.tensor_copy(qT[p][:, tsl], pq[:])

            def attn_chunk(p, qc):
                qsl = slice(qc * 512, (qc + 1) * 512)
                av = [ps_av.tile([65, 512], f32, tag=f"av{h}",
                                 name=f"av{h}_{p}_{qc}") for h in range(2)]
                for kb in range(KB):
                    s_ps = ps_s.tile([128, 1024], f32, tag="s",
                                     name=f"s_{p}_{qc}_{kb}")
                    nc.tensor.matmul(
                        s_ps[:, 0:512],
                        kT[p][0:64, kb * 128:(kb + 1) * 128],
                        qT[p][0:64, qsl], start=True, stop=True)
                    nc.tensor.matmul(
                        s_ps[:, 512:1024],
                        kT[p][64:128, kb * 128:(kb + 1) * 128],
                        qT[p][64:128, qsl], start=True, stop=True)
                    e = sb_e.tile([128, 1024], fr, tag="e",
                                  name=f"e_{p}_{qc}_{kb}")
                    nc.scalar.activation(e[:], s_ps[:], Exp, scale=0.125)
                    for h in range(2):
                        hh = (2 * p + h) * 65
                        nc.tensor.matmul(
                            av[h][:], tv[:, kb, hh:hh + 65],
                            e[:, h * 512:(h + 1) * 512],
                            start=(kb == 0), stop=(kb == KB - 1))
                # evacuate AV (frees the accumulator banks fast): o rows
                # unnormalized into onorm, rowsum rows into the recip tile
                rc, ro = rcin[qc % 2], rcout[qc % 2]
                for h in range(2):
                    nc.vector.tensor_copy(
                        onorm[p][h * 64:(h + 1) * 64, qsl], av[h][0:64, :])
                    nc.vector.tensor_copy(
                        rc[32 * h:32 * h + 1, :], av[h][64:65, :])
                # normalization, off the AV critical path
                with nc.allow_low_precision(reason="softmax denom recip"):
                    nc.vector.reciprocal_approx_fast(ro[0:33, :], rc[0:33, :])
                r_ps = ps_a.tile([128, 512], f32, tag="qkv",
                                 name=f"rps_{p}_{qc}")
                nc.tensor.matmul(r_ps[:], onesbd[:], ro[:],
                                 start=True, stop=True)
                r_sb = sb_r.tile([128, 512], f32, tag="r_sb",
                                 name=f"rsb_{p}_{qc}")
                nc.vector.tensor_copy(r_sb[:], r_ps[:])
                for h in range(2):
                    hsl = slice(h * 64, (h + 1) * 64)
                    nc.vector.tensor_mul(
                        onorm[p][hsl, qsl], onorm[p][hsl, qsl], r_sb[hsl, :])

            def proj_block(qb):
                bsl = slice(qb * 128, (qb + 1) * 128)
                so = sb_out.tile([128, 1024], f32, tag="so", name=f"so{qb}")
                for cc in range(2):
                    csl = slice(cc * 512, (cc + 1) * 512)
                    pp = ps_a.tile([128, 512], f32, tag="qkv",
                                   name=f"pp{qb}_{cc}")
                    nc.tensor.matmul(pp[:], onorm[0][:, bsl], twp[:, 0, csl],
                                     start=True, stop=False)
                    nc.tensor.matmul(pp[:], onorm[1][:, bsl], twp[:, 1, csl],
                                     start=False, stop=True)
                    nc.vector.tensor_copy(so[:, csl], pp[:])
                nc.sync.dma_start(out=out[bsl, :], in_=so[:])

            # ---- schedule ----
            q_proj(0)
            for t in range(TC):
                kv_chunk(t)
            attn_chunk(0, 0)
            for qc in range(1, QC):
                q_proj(qc)
                attn_chunk(0, qc)
            for qc in range(QC):
                attn_chunk(1, qc)
                for qb in range(4 * qc, 4 * qc + 4):
                    proj_block(qb)

    nc.finalize()
    return nc


def bass_ap_heads(tv, kb):
    # view of tv[:, kb, :] as [128, HPC, 64] hitting cols h*65..h*65+63
    return tv[:, kb, :].rearrange("p (h s) -> p h s", h=HPC)[:, :, 0:64]


def _shard_inputs(x, y, Wq, Wkv, Wp):
    x = np.asarray(x, dtype=np.float32)
    y = np.asarray(y, dtype=np.float32)
    Wq = np.asarray(Wq, dtype=np.float32)
    Wkv = np.asarray(Wkv, dtype=np.float32)
    Wp = np.asarray(Wp, dtype=np.float32)
    bd = np.zeros((64, 128), dtype=np.float32)
    bd[0, 0:64] = 1.0
    bd[32, 64:128] = 1.0
    in_maps = []
    for core in range(NCORES):
        b, g = divmod(core, G)
        sl = slice(g * HPC * D, (g + 1) * HPC * D)
        in_maps.append({
            "xT": np.ascontiguousarray(x[b].T),
            "yT": np.ascontiguousarray(y[b].T),
            "wq": np.ascontiguousarray(Wq[sl, :].T),
            "wk": np.ascontiguousarray(Wkv[sl, :].T),
            "wv": np.ascontiguousarray(Wkv[C:][sl, :].T),
            "wp": np.ascontiguousarray(Wp[:, sl].T),
            "bd": bd,
        })
    return in_maps


def kernel(x, y, Wq, Wkv, Wp, bp):
    from concourse.bass_utils import run_bass_kernel_spmd

    if "nc" not in _CACHE:
        _CACHE["nc"] = _build()
    nc = _CACHE["nc"]

    in_maps = _shard_inputs(x, y, Wq, Wkv, Wp)
    res = run_bass_kernel_spmd(nc, in_maps, core_ids=list(range(NCORES)))

    bp = np.asarray(bp, dtype=np.float32)
    full = np.zeros((B, N, C), dtype=np.float32)
    for core in range(NCORES):
        b = core // G
        full[b] += res.results[core]["out"]
    full += bp[None, None, :]
    return full
